# revision 31
# baseline (speedup 1.0000x reference)
"""Trainium2 Bass kernel for nn_Complex_Loss (complex regression loss).

Reference semantics (see problem):
    D = 4096; out/g_t: [B=16384, 2D=8192] f32, first half real, second imag.
    err = g_t - out ; sq = err_r^2 + err_i^2            [B, D]
    e_max = max_j sqrt(sq)                              [B]
    correct = argmax(gt_r) == argmax(out_r)             [B]
    masked = correct & (e_max < e_thresh)
    sum_sq = sum over rows not masked of sum_j sq
    count = 1 + sum(masked ? 1 : D)
    return sum_sq / count

Strategy: data-parallel over the batch axis across 8 NeuronCores
(2048 rows per core).  The kernel is memory-bound: it must stream
2 x 2048 x 8192 x 4B = 134 MB per core, so everything is built around
keeping the DMA queues saturated:

  - per 128-row tile, four 2 MB DMAs (one per tensor-half), alternated
    across the two independent HWDGE queues (qSPDynamicHW via nc.sync,
    qActDynamicHW via nc.scalar) so the queues' per-DMA overheads overlap
    at SDMA packet granularity (measured ~7 us faster than one queue);
  - err = g - o in fp16 on DVE (GpSimd subs measured slower/noisier);
  - ACT Square(err) with accum_out produces the per-row sum of squares;
  - per-row stats accumulate in one SBUF tile; the bulk store is issued
    early on the idle SWDGE queue, only a tiny tail store remains at the
    pipeline drain;
  - the last tile's LOADS are split into 1 MB pieces and its compute is
    "tapered" into 4x1024-col chunks so the drain after the final DMA
    byte is short (load_taper=True; measured ~5 us over no-load-taper).

Tuning notes (2026-08-10 session, all interleaved-slope measured;
process-to-process drift is +-3-5 us, so rankings were taken from
drift-cancelled round-robin runs within one process):
pure-DMA floor ~388 us (346 GB/s/core, ~96% of the 2.9 TB/s chip HBM
peak shared by 8 cores); the full-tile phase is clean (no-tail-compute
build = 389.3 us), so only the last tile's drain is recoverable.  The
default config (2x1MB tail load pieces, compute chunks
(1024,1024,1792,256), tail store triggered from ACT's own stream
'scalar' right after the final square -- no cross-engine sem hop)
measured ~390-391 us and beat: 4x1024 chunks + sync store (392.6),
finer tail chunks 6-8 (397-402), finer load pieces 4x512KB (397),
split tail store (393.5), paired g/o load order, 3-queue loads (434),
gpsimd tail store (394.3).  Each DMA-completion boundary that GATES
compute costs ~2 us of completion-receipt latency under full HBM load
(invisible in DMA-only probes), and each extra drain-path op costs
~0.4-0.5 us of sem/wake overhead -- both push toward a coarse tail
with only the FINAL chunk small (256 cols shrinks the last
sub+square on the drain path).

Masking: e_max^2 = max_j sq_j >= rowsum/D, so any row with
rowsum >= D*e_thresh^2 is provably unmasked (for randn-scale data that
is every row by a ~38-sigma margin).  The astronomically-rare remainder
is recomputed exactly on the host from the full inputs, so kernel() is
correct for arbitrary inputs, not just the graded distribution.
"""
import numpy as np
from contextlib import ExitStack

import concourse.bass as bass
import concourse.tile as tile
from concourse import bacc, mybir
from concourse.bass_utils import run_bass_kernel_spmd

# Problem shape (hardcoded per the task contract).
B, TWO_D = 16384, 8192
D = TWO_D // 2            # 4096
N_CORES = 8
R = B // N_CORES          # 2048 rows per core
P = 128                   # SBUF partitions
NT = R // P               # 16 row-tiles per core

f32 = mybir.dt.float32
f16 = mybir.dt.float16
Alu = mybir.AluOpType
Act = mybir.ActivationFunctionType

# stats columns: [0:NT]    sum_r   (row sum of err_r^2, per tile)
#                [NT:2NT]  sum_i
# The LAST tile is "tapered" into TC fine chunks (short drain tail); its
# row sums land in [2NT : 2NT+2*TC) instead of cols NT-1 / 2NT-1.
TC = 4                    # taper chunks per half (last tile)
NSF = 2 * NT + 2 * TC     # 40

_NC_CACHE = {}


def _build_nc(n_loop=0, o_engine="sync", compute=True, split=2, taper=True,
              sub_engine="dve", store_split=True, load_taper=True,
              tail_store="scalar", load_pieces=(2048, 2048),
              taper_widths=(1024, 1024, 1792, 256),
              split_tail_store=False, tail_ring="tensor"):
    """Build the per-core program.  n_loop>0 wraps the whole body in a
    hardware For_i loop (used only for timing measurements).  compute=False
    and split are experiment knobs (DMA-only builds, loads split across
    both HWDGE queues).  taper chops the LAST tile's compute into fine
    chunks so the pipeline drain tail after the final DMA is short."""
    tws = taper_widths
    tcn = len(tws)
    nsf = 2 * NT + 2 * tcn
    nc = bacc.Bacc("TRN2", target_bir_lowering=False, debug=False,
                   num_devices=N_CORES)
    g = nc.dram_tensor("g", [R, TWO_D], f32, kind="ExternalInput").ap()
    o = nc.dram_tensor("o", [R, TWO_D], f32, kind="ExternalInput").ap()
    stats = nc.dram_tensor("stats", [P, nsf], f32, kind="ExternalOutput").ap()

    o_dma = {"scalar": nc.scalar, "sync": nc.sync}[o_engine]

    with tile.TileContext(nc) as tc, ExitStack() as ctx:
        iop = ctx.enter_context(tc.tile_pool(name="io", bufs=2))
        ep = ctx.enter_context(tc.tile_pool(name="err", bufs=2))
        dp = ctx.enter_context(tc.tile_pool(name="dummy", bufs=1))
        sp = ctx.enter_context(tc.tile_pool(name="st", bufs=1))

        def body():
            stf = sp.tile([P, nsf], f32, tag="stf")
            # dummy Square output, written every tile and never read
            dum = dp.tile([P, D], f16, tag="dum")

            # Software-pipeline skew: the ACT Squares of tile t are
            # emitted after the o-DMA trigger of tile t+1 so ACT's
            # in-order stream never blocks the next load behind a
            # compute-dependency wait.
            pending = []  # (t, err_r, err_i)

            def flush_pending():
                t, err_r, err_i = pending.pop(0)
                nc.scalar.activation(dum[:], err_r[:], Act.Square,
                                     accum_out=stf[:, t:t + 1])
                nc.scalar.activation(dum[:], err_i[:], Act.Square,
                                     accum_out=stf[:, NT + t:NT + t + 1])

            if not compute:
                nc.vector.memset(stf[:], 0.0)
            elif taper:
                # cols NT-1 / 2NT-1 are unused when the last tile tapers
                nc.vector.memset(stf[:, NT - 1:NT], 0.0)
                nc.vector.memset(stf[:, 2 * NT - 1:2 * NT], 0.0)

            for t in range(NT):
                r0 = t * P
                gt_ = iop.tile([P, TWO_D], f32, tag="g")
                ot_ = iop.tile([P, TWO_D], f32, tag="o")
                if load_taper and taper and t == NT - 1 and split == 2:
                    # Last tile: sub-MB load pieces so tail compute can
                    # start as each piece lands.  tail_ring="cross": each
                    # ring carries one half of each tensor (final pieces
                    # gate r AND i tails).  "tensor": one ring per tensor,
                    # r pieces first -- both rings' FINAL pieces gate the
                    # SAME i-half columns, halving the drain-gated work.
                    if tail_ring == "tensor":
                        ring_plan = ((nc.sync, nc.sync, o, ot_),
                                     (nc.scalar, nc.scalar, g, gt_))
                    else:
                        ring_plan = ((nc.sync, nc.scalar, g, gt_),
                                     (nc.scalar, nc.sync, o, ot_))
                    for q0, q1, src_t, dst in ring_plan:
                        a = 0
                        for H in load_pieces:
                            q0.dma_start(dst[:, a:a + H],
                                         src_t[r0:r0 + P, a:a + H])
                            a += H
                        a = D
                        for H in load_pieces:
                            q1.dma_start(dst[:, a:a + H],
                                         src_t[r0:r0 + P, a:a + H])
                            a += H
                elif split == 1:
                    nc.sync.dma_start(gt_[:], g[r0:r0 + P, :])
                    o_dma.dma_start(ot_[:], o[r0:r0 + P, :])
                else:
                    nc.sync.dma_start(gt_[:, 0:D], g[r0:r0 + P, 0:D])
                    nc.scalar.dma_start(gt_[:, D:TWO_D], g[r0:r0 + P, D:TWO_D])
                    nc.scalar.dma_start(ot_[:, 0:D], o[r0:r0 + P, 0:D])
                    nc.sync.dma_start(ot_[:, D:TWO_D], o[r0:r0 + P, D:TWO_D])

                if not compute:
                    continue

                if pending:
                    flush_pending()

                if taper and t == NT - 1:
                    while pending:
                        flush_pending()
                    if store_split:
                        # Bulk store (tiles 0..NT-2) early, on the idle
                        # SWDGE queue: off the critical drain tail.
                        nc.gpsimd.dma_start(stats[:, 0:2 * NT],
                                            stf[:, 0:2 * NT])
                    # Last tile: fine-grained chunks so the final
                    # sub->square->store chain after the last DMA is short.
                    c0 = 0
                    for c, CW in enumerate(tws):
                        sub_i = (nc.vector if sub_engine == "dve"
                                 else nc.gpsimd)
                        er = dp.tile([P, CW], f16, tag=f"ler{c % 2}")
                        nc.vector.tensor_sub(er[:], gt_[:, c0:c0 + CW],
                                             ot_[:, c0:c0 + CW])
                        ei = dp.tile([P, CW], f16, tag=f"lei{c % 2}")
                        sub_i.tensor_sub(ei[:], gt_[:, D + c0:D + c0 + CW],
                                         ot_[:, D + c0:D + c0 + CW])
                        nc.scalar.activation(dum[:, 0:CW], er[:], Act.Square,
                                             accum_out=stf[:, 2 * NT + c:
                                                           2 * NT + c + 1])
                        last = split_tail_store and store_split and c == tcn - 1
                        if last:
                            # all tail cols except the final i-chunk are
                            # ready; store them from the parked sync ring
                            nc.sync.dma_start(stats[:, 2 * NT:nsf - 1],
                                              stf[:, 2 * NT:nsf - 1])
                        nc.scalar.activation(dum[:, 0:CW], ei[:], Act.Square,
                                             accum_out=stf[:, 2 * NT + tcn + c:
                                                           2 * NT + tcn + c + 1])
                        if last:
                            # final column: trigger sits in ACT's own stream
                            # right after the square -- no cross-engine hop
                            nc.scalar.dma_start(stats[:, nsf - 1:nsf],
                                                stf[:, nsf - 1:nsf])
                        c0 += CW
                    continue

                # err = g - o in fp16 (feeds ACT's Square+rowsum)
                sub_i = nc.vector if sub_engine == "dve" else nc.gpsimd
                err_r = ep.tile([P, D], f16, tag="err_r")
                nc.vector.tensor_sub(err_r[:], gt_[:, 0:D], ot_[:, 0:D])
                err_i = ep.tile([P, D], f16, tag="err_i")
                sub_i.tensor_sub(err_i[:], gt_[:, D:TWO_D],
                                 ot_[:, D:TWO_D])
                pending.append((t, err_r, err_i))

            while pending:
                flush_pending()
            if compute and taper and store_split:
                if not split_tail_store:
                    st_q = {"gpsimd": nc.gpsimd, "sync": nc.sync,
                            "scalar": nc.scalar}[tail_store]
                    st_q.dma_start(stats[:, 2 * NT:nsf], stf[:, 2 * NT:nsf])
            else:
                nc.sync.dma_start(stats[:, :], stf[:])

        if n_loop > 0:
            with tc.For_i(0, n_loop, 1) as _i:
                body()
        else:
            body()

    nc.compile()
    return nc


# ---------------------------------------------------------------------------
# v2: same data-parallel streaming design, but the LAST row-tile is processed
# as fine column chunks whose loads are interleaved with their compute, so the
# sub->square->store chain after the final DMA byte is ~3 us instead of ~13.
# Stats layout v2: 15 full tiles (cols [0:15) r / [15:30) i) + len(TAILW)
# chunk columns per half for the last tile ([30:30+NTC) r, then i).
TAILW = (2048, 1024, 512, 512)              # last-tile column chunk widths
NF = NT - 1                                  # 15 full row-tiles
NTC = len(TAILW)
NSF2 = 2 * NF + 2 * NTC


def _build_nc2(n_loop=0, compute=True, store=True, nq=2, split=2,
               tail_ranges=None, tail_order="paired", tail_store="scalar",
               tail_compute=True):
    """v2 per-core program.  compute/store/nq/split are probe knobs:
    compute=False -> DMA-only build; store=False -> no stats stores;
    nq=3 -> spread loads over sync+scalar+gpsimd; split=4 -> 1MB pieces.
    tail_order: "paired" = per column range, g and o pieces issued together
    on opposite queues (compute trails the stream range by range);
    "tmajor" = all g pieces then all o pieces (v1 load_taper style).
    tail_compute=False skips the last tile's compute (timing probe only)."""
    ranges = TAILW if tail_ranges is None else tail_ranges
    ntc = len(ranges)
    nsf = 2 * NF + 2 * ntc
    nc = bacc.Bacc("TRN2", target_bir_lowering=False, debug=False,
                   num_devices=N_CORES)
    g = nc.dram_tensor("g", [R, TWO_D], f32, kind="ExternalInput").ap()
    o = nc.dram_tensor("o", [R, TWO_D], f32, kind="ExternalInput").ap()
    stats = nc.dram_tensor("stats", [P, nsf], f32, kind="ExternalOutput").ap()

    with tile.TileContext(nc) as tc, ExitStack() as ctx:
        iop = ctx.enter_context(tc.tile_pool(name="io", bufs=2))
        ep = ctx.enter_context(tc.tile_pool(name="err", bufs=2))
        dp = ctx.enter_context(tc.tile_pool(name="dummy", bufs=1))
        sp = ctx.enter_context(tc.tile_pool(name="st", bufs=1))

        def body():
            stf = sp.tile([P, nsf], f32, tag="stf")
            dum = dp.tile([P, D], f16, tag="dum")
            pending = []  # (t, err_r, err_i): ACT squares trail by one tile

            def flush_pending():
                t, err_r, err_i = pending.pop(0)
                nc.scalar.activation(dum[:], err_r[:], Act.Square,
                                     accum_out=stf[:, t:t + 1])
                nc.scalar.activation(dum[:], err_i[:], Act.Square,
                                     accum_out=stf[:, NF + t:NF + t + 1])

            if not compute:
                nc.vector.memset(stf[:], 0.0)

            for t in range(NF):
                r0 = t * P
                gt_ = iop.tile([P, TWO_D], f32, tag="g")
                ot_ = iop.tile([P, TWO_D], f32, tag="o")
                if split == 4:
                    H = D // 2
                    for k in range(4):
                        a = k * H
                        (nc.sync, nc.scalar)[k % 2].dma_start(
                            gt_[:, a:a + H], g[r0:r0 + P, a:a + H])
                    for k in range(4):
                        a = k * H
                        (nc.scalar, nc.sync)[k % 2].dma_start(
                            ot_[:, a:a + H], o[r0:r0 + P, a:a + H])
                elif split == 5:
                    # paired 512KB pieces (stream-order probe): alternate g/o
                    # per 1024-col range within each ring
                    H = D // 4
                    for k in range(4):
                        a = k * H
                        ai = D + a
                        nc.sync.dma_start(gt_[:, a:a + H], g[r0:r0 + P, a:a + H])
                        nc.scalar.dma_start(ot_[:, a:a + H],
                                            o[r0:r0 + P, a:a + H])
                        nc.scalar.dma_start(gt_[:, ai:ai + H],
                                            g[r0:r0 + P, ai:ai + H])
                        nc.sync.dma_start(ot_[:, ai:ai + H],
                                          o[r0:r0 + P, ai:ai + H])
                elif nq == 3:
                    nc.sync.dma_start(gt_[:, 0:D], g[r0:r0 + P, 0:D])
                    nc.scalar.dma_start(gt_[:, D:TWO_D], g[r0:r0 + P, D:TWO_D])
                    nc.gpsimd.dma_start(ot_[:, 0:D], o[r0:r0 + P, 0:D])
                    (nc.sync, nc.scalar, nc.gpsimd)[t % 3].dma_start(
                        ot_[:, D:TWO_D], o[r0:r0 + P, D:TWO_D])
                else:
                    nc.sync.dma_start(gt_[:, 0:D], g[r0:r0 + P, 0:D])
                    nc.scalar.dma_start(gt_[:, D:TWO_D], g[r0:r0 + P, D:TWO_D])
                    nc.scalar.dma_start(ot_[:, 0:D], o[r0:r0 + P, 0:D])
                    nc.sync.dma_start(ot_[:, D:TWO_D], o[r0:r0 + P, D:TWO_D])

                if not compute:
                    continue
                if pending:
                    flush_pending()
                err_r = ep.tile([P, D], f16, tag="err_r")
                nc.vector.tensor_sub(err_r[:], gt_[:, 0:D], ot_[:, 0:D])
                err_i = ep.tile([P, D], f16, tag="err_i")
                nc.vector.tensor_sub(err_i[:], gt_[:, D:TWO_D],
                                     ot_[:, D:TWO_D])
                pending.append((t, err_r, err_i))

            # ---- last row-tile: fine column chunks, loads interleaved ----
            r0 = NF * P
            gt_ = iop.tile([P, TWO_D], f32, tag="g")
            ot_ = iop.tile([P, TWO_D], f32, tag="o")
            if tail_order == "paired":
                c0 = 0
                for W in ranges:
                    a, b = c0, c0 + W
                    ai, bi = D + c0, D + c0 + W
                    nc.sync.dma_start(gt_[:, a:b], g[r0:r0 + P, a:b])
                    nc.scalar.dma_start(ot_[:, a:b], o[r0:r0 + P, a:b])
                    nc.scalar.dma_start(gt_[:, ai:bi], g[r0:r0 + P, ai:bi])
                    nc.sync.dma_start(ot_[:, ai:bi], o[r0:r0 + P, ai:bi])
                    c0 += W
            else:  # tmajor: all g pieces, then all o pieces
                for q0, q1, src_t, dst in ((nc.sync, nc.scalar, g, gt_),
                                           (nc.scalar, nc.sync, o, ot_)):
                    c0 = 0
                    for W in ranges:
                        q0.dma_start(dst[:, c0:c0 + W],
                                     src_t[r0:r0 + P, c0:c0 + W])
                        q1.dma_start(dst[:, D + c0:D + c0 + W],
                                     src_t[r0:r0 + P, D + c0:D + c0 + W])
                        c0 += W

            if not compute:
                if store:
                    nc.sync.dma_start(stats[:, :], stf[:])
                return
            while pending:
                flush_pending()
            if store:
                # bulk store (full tiles) early, on the idle SWDGE queue
                nc.gpsimd.dma_start(stats[:, 0:2 * NF], stf[:, 0:2 * NF])
            if not tail_compute:
                nc.vector.memset(stf[:, 2 * NF:nsf], 0.0)
                if store:
                    nc.gpsimd.dma_start(stats[:, 2 * NF:nsf],
                                        stf[:, 2 * NF:nsf])
                return
            st_q = {"scalar": nc.scalar, "sync": nc.sync,
                    "gpsimd": nc.gpsimd}[tail_store]
            c0 = 0
            for c, W in enumerate(ranges):
                er = dp.tile([P, W], f16, tag=f"ter{c % 2}")
                nc.vector.tensor_sub(er[:], gt_[:, c0:c0 + W],
                                     ot_[:, c0:c0 + W])
                ei = dp.tile([P, W], f16, tag=f"tei{c % 2}")
                nc.vector.tensor_sub(ei[:], gt_[:, D + c0:D + c0 + W],
                                     ot_[:, D + c0:D + c0 + W])
                nc.scalar.activation(dum[:, 0:W], er[:], Act.Square,
                                     accum_out=stf[:, 2 * NF + c:
                                                   2 * NF + c + 1])
                if store and c == ntc - 1:
                    # everything except the very last column is ready now
                    st_q.dma_start(stats[:, 2 * NF:nsf - 1],
                                   stf[:, 2 * NF:nsf - 1])
                nc.scalar.activation(dum[:, 0:W], ei[:], Act.Square,
                                     accum_out=stf[:, 2 * NF + ntc + c:
                                                   2 * NF + ntc + c + 1])
                c0 += W
            if store:
                st_q.dma_start(stats[:, nsf - 1:nsf], stf[:, nsf - 1:nsf])

        if n_loop > 0:
            with tc.For_i(0, n_loop, 1) as _i:
                body()
        else:
            body()

    nc.compile()
    return nc


def combine_stats2(stats, epoch, out=None, g_t=None):
    """Host-side tail for the v2 stats layout (see _build_nc2)."""
    stats = stats.reshape(N_CORES, P, NSF2).astype(np.float64)
    sum_r = np.concatenate(
        [stats[:, :, 0:NF],
         stats[:, :, 2 * NF:2 * NF + NTC].sum(axis=2, keepdims=True)], axis=2)
    sum_i = np.concatenate(
        [stats[:, :, NF:2 * NF],
         stats[:, :, 2 * NF + NTC:NSF2].sum(axis=2, keepdims=True)], axis=2)
    rowsum = (sum_r + sum_i).transpose(0, 2, 1).reshape(-1)
    return _finish(rowsum, epoch, out, g_t)


def _finish(rowsum, epoch, out, g_t):
    """Shared host tail: bound-based mask check + exact fallback."""
    thresh = _e_thresh(epoch)
    margin = 1.01
    suspect = rowsum < D * (thresh * margin) ** 2
    total = rowsum.sum()
    count = 1.0 + rowsum.size * float(D)
    if suspect.any():
        assert out is not None and g_t is not None, (
            "suspect rows require the full inputs for exact recomputation")
        idx = np.nonzero(suspect)[0]
        gt_s = np.asarray(g_t[idx], dtype=np.float32)
        ot_s = np.asarray(out[idx], dtype=np.float32)
        err = gt_s - ot_s
        sq = err[:, :D] ** 2 + err[:, D:] ** 2
        e_max = np.sqrt(sq.max(axis=1))
        correct = gt_s[:, :D].argmax(axis=1) == ot_s[:, :D].argmax(axis=1)
        masked = correct & (e_max < thresh)
        exact_rowsum = sq.astype(np.float64).sum(axis=1)
        total += (np.where(masked, 0.0, exact_rowsum) - rowsum[idx]).sum()
        count += float(masked.sum()) * (1.0 - float(D))
    return np.float32(total / count)


def get_nc():
    if "nc" not in _NC_CACHE:
        _NC_CACHE["nc"] = _build_nc()
    return _NC_CACHE["nc"]


def _e_thresh(epoch):
    E_T_INIT = 0.5
    if int(epoch) % 10 == 0:
        return np.float32(E_T_INIT * np.exp(-0.2))
    return np.float32(E_T_INIT)


def combine_stats(stats, epoch, out=None, g_t=None, taper=True):
    """Host-side tail: per-core [P, NSF] stats (concatenated on axis 0 to
    [N_CORES*P, NSF]) -> scalar loss.

    Row (core c, tile t, partition p) = c*R + t*P + p maps to
    stats[c*P + p, col + t] for col in {0, NT}; with taper the last
    tile's row sums live in cols [2NT : 2NT+2*TC) instead.
    """
    nsf = stats.shape[-1]
    tcn = (nsf - 2 * NT) // 2           # taper chunk count (from layout)
    stats = stats.reshape(N_CORES, P, nsf)
    sum_r = stats[:, :, 0:NT].copy()    # [C, P, NT]
    sum_i = stats[:, :, NT:2 * NT].copy()
    if taper:
        tp = stats[:, :, 2 * NT:2 * NT + 2 * tcn].astype(np.float64)
        sum_r[:, :, NT - 1] = tp[:, :, 0:tcn].sum(axis=2)
        sum_i[:, :, NT - 1] = tp[:, :, tcn:2 * tcn].sum(axis=2)

    # -> [C, NT, P] -> flat row order (c, t, p)
    rowsum = (sum_r + sum_i).astype(np.float64).transpose(0, 2, 1).reshape(-1)

    thresh = _e_thresh(epoch)
    # rowsum underestimates the true sum only up to fp16 rounding of err
    # (~2^-11 relative); inflate the suspect margin accordingly.  A row is
    # suspect only if sqrt(rowsum/D) fails to clear the threshold.
    margin = 1.01
    suspect = rowsum < D * (thresh * margin) ** 2

    total = rowsum.sum()
    count = 1.0 + rowsum.size * float(D)
    if suspect.any():
        # Exact recomputation for rows the device bound cannot clear.
        assert out is not None and g_t is not None, (
            "suspect rows require the full inputs for exact recomputation")
        idx = np.nonzero(suspect)[0]
        gt_s = np.asarray(g_t[idx], dtype=np.float32)
        ot_s = np.asarray(out[idx], dtype=np.float32)
        err = gt_s - ot_s
        sq = err[:, :D] ** 2 + err[:, D:] ** 2
        e_max = np.sqrt(sq.max(axis=1))
        correct = gt_s[:, :D].argmax(axis=1) == ot_s[:, :D].argmax(axis=1)
        masked = correct & (e_max < thresh)
        exact_rowsum = sq.astype(np.float64).sum(axis=1)
        # replace the device rowsum with the exact one for suspect rows;
        # masked rows contribute 0 to sum_sq and swap D -> 1 in count.
        total += (np.where(masked, 0.0, exact_rowsum) - rowsum[idx]).sum()
        count += float(masked.sum()) * (1.0 - float(D))
    return np.float32(total / count)


def kernel(out, g_t, epoch):
    out = np.asarray(out, dtype=np.float32)
    g_t = np.asarray(g_t, dtype=np.float32)
    assert out.shape == (B, TWO_D) and g_t.shape == (B, TWO_D)

    nc = get_nc()
    in_maps = [{"g": g_t[c * R:(c + 1) * R], "o": out[c * R:(c + 1) * R]}
               for c in range(N_CORES)]
    res = run_bass_kernel_spmd(nc, in_maps, list(range(N_CORES))).results
    stats = np.concatenate([res[c]["stats"] for c in range(N_CORES)], axis=0)
    return combine_stats(stats, epoch, out=out, g_t=g_t)

